# revision 9
# baseline (speedup 1.0000x reference)
"""Trainium2 Bass kernel for nn_DfOpCoefLoop (deep-filter complex FIR + alpha blend).

Reference semantics (per batch b, time t, freq bin f < 96):
    spec_f[t,f] = sum_{i=0..4} x[t+i-2, f] * coefs[t,i,f]      (complex MAC, zero-padded in t)
    out[t,f]    = alpha[t] * spec_f[t,f] + (1-alpha[t]) * x[t,f]
    out[t,f]    = spec[t,f]                                    (f >= 96 passthrough)

Device-side transformations:
  - Alpha folding (host): c' = alpha*c with (1-alpha) added to the real coef of
    the center tap (i=2) -> pure 5-tap complex FIR on device.
  - Layout: rows = (batch, freq) pairs -> 384 rows/core = 3 tiles of 128
    partitions; time along the free dim; tap shifts are free-dim offsets.
  - fp16 TensorTensor everywhere (2x_1p mode, 0.52 ns/elem on DVE).
  - Karatsuba 3-mult complex tap product:
        m1_i = xr_i*cr_i   m2_i = xi_i*ci_i   m3_i = (xr_i+xi_i)*(cr_i+ci_i)
        re = R1 - R2       im = R3 - R1 - R2      (R_g = sum_i m_g_i)
  - Products as big [128, taps, 1000] TensorTensor instrs via overlapping APs.
  - Tap sums via a log tree of TensorTensor adds over all 3 groups at once.
  - GPSIMD (2.4 ns/elem) takes the early off-critical-path work: s = xr+xi and
    m2 taps 0..3; DVE does the rest.
  - Software pipelining: the tree/combines of row-tile rt-1 are emitted after
    the products of row-tile rt, hiding GPSIMD/DMA latency.
  - DMAs are split in halves across the two issue queues (sync/scalar) in
    need-order (x, cr, ci, cs) so the first products start ~5us in.
"""

import numpy as np

ORDER = 5
LOOKAHEAD = 2
F = 96
T = 1000
TP = T + ORDER - 1          # 1004 padded time samples
B = 32
NCORES = 8
BPC = B // NCORES           # 4 batches per core
ROWS = BPC * F              # 384 rows per core
NRT = ROWS // 128           # 3 row-tiles per core

_CACHE = {}


def _build_program():
    import concourse.bacc as bacc
    import concourse.mybir as mybir
    import concourse.tile as tile
    from concourse.ap import AP

    nc = bacc.Bacc("TRN2", target_bir_lowering=False, debug=False)
    dt = mybir.dt.float16
    mul = mybir.AluOpType.mult

    x_t = nc.dram_tensor("x_t", [NRT, 128, 2 * TP], dt, kind="ExternalInput").ap()
    cx_t = nc.dram_tensor("cx_t", [NRT, 128, 15 * T], dt, kind="ExternalInput").ap()
    out_t = nc.dram_tensor("out_t", [NRT, 128, 2 * T], dt, kind="ExternalOutput").ap()

    def taps(tile_ap, row_elems, off, n):
        """Overlapping [128, n, T] view: (p, i, t) -> base + off + i + t."""
        return AP(tile_ap.tensor, tile_ap.offset + off, [[row_elems, 128], [1, n], [1, T]])

    with tile.TileContext(nc) as tc:
        with (
            tc.tile_pool(name="xp", bufs=2) as xp,
            tc.tile_pool(name="sp", bufs=2) as sp,
            tc.tile_pool(name="cp", bufs=2) as cp,
            tc.tile_pool(name="pp", bufs=1) as pp,
            tc.tile_pool(name="ap_", bufs=1) as ap_,
            tc.tile_pool(name="bp", bufs=1) as bp,
            tc.tile_pool(name="rp", bufs=1) as rp,
            tc.tile_pool(name="obp", bufs=2) as obp,
        ):
            state = {}

            def products(rt):
                x = xp.tile([128, 2 * TP], dt, name="x")
                s = sp.tile([128, TP], dt, name="s")
                c = cp.tile([128, 15, T], dt, name="c")
                # halve x and cr across both queues: FIFO per queue makes the
                # engines finish cr before starting ci/cs (critical-path first)
                H = 64
                nc.sync.dma_start(x[:H], x_t[rt, :H])
                nc.scalar.dma_start(x[H:], x_t[rt, H:])
                nc.sync.dma_start(c[:H, 0:5], cx_t[rt, :H, : 5 * T])
                nc.scalar.dma_start(c[H:, 0:5], cx_t[rt, H:, : 5 * T])
                nc.sync.dma_start(c[:, 5:10], cx_t[rt, :, 5 * T : 10 * T])
                nc.scalar.dma_start(c[:, 10:15], cx_t[rt, :, 10 * T :])

                p = pp.tile([128, 3, ORDER, T], dt, name="p")
                # all-DVE: concurrent GPSIMD work poisons DVE throughput
                # (SBUF contention), so everything stays on the vector engine
                nc.vector.tensor_add(s[:], x[:, 0:TP], x[:, TP : 2 * TP])
                nc.vector.tensor_tensor(p[:, 0], taps(x[:], 2 * TP, 0, 5), c[:, 0:5], op=mul)
                nc.vector.tensor_tensor(p[:, 1], taps(x[:], 2 * TP, TP, 5), c[:, 5:10], op=mul)
                nc.vector.tensor_tensor(p[:, 2], taps(s[:], TP, 0, 5), c[:, 10:15], op=mul)
                state[rt] = p

            def tree(rt):
                p = state.pop(rt)
                a = ap_.tile([128, 3, 2, T], dt, name="a")
                bb = bp.tile([128, 3, T], dt, name="bb")
                r = rp.tile([128, 3, T], dt, name="r")
                ti = rp.tile([128, T], dt, name="ti")
                ob = obp.tile([128, 2 * T], dt, name="ob")
                nc.vector.tensor_add(a[:], p[:, :, 0:2], p[:, :, 2:4])
                nc.vector.tensor_add(bb[:], a[:, :, 0], a[:, :, 1])
                nc.vector.tensor_add(r[:], bb[:], p[:, :, 4])
                nc.vector.tensor_sub(ob[:, 0:T], r[:, 0], r[:, 1])
                nc.sync.dma_start(out_t[rt, :, 0:T], ob[:, 0:T])
                nc.vector.tensor_sub(ti[:], r[:, 2], r[:, 0])
                nc.vector.tensor_sub(ob[:, T : 2 * T], ti[:], r[:, 1])
                nc.scalar.dma_start(out_t[rt, :, T : 2 * T], ob[:, T : 2 * T])

            for rt in range(NRT):
                products(rt)
                tree(rt)
    nc.compile()
    return nc


def _get_program():
    if "nc" not in _CACHE:
        _CACHE["nc"] = _build_program()
    return _CACHE["nc"]


def _host_prep(spec, coefs, alpha):
    """Build the (row, free) fp16 device layouts for all 32 batches at once."""
    x = np.asarray(spec[:, 0, :, :F, :], dtype=np.float32)      # (B, T, F, 2)
    X = np.zeros((B, F, 2, TP), np.float16)
    X[:, :, :, LOOKAHEAD : LOOKAHEAD + T] = x.transpose(0, 2, 3, 1)
    X = X.reshape(B * F, 2 * TP)

    a = np.asarray(alpha, dtype=np.float32)[:, :, 0]            # (B, T)
    cc = np.asarray(coefs, dtype=np.float32) * a[:, :, None, None, None]
    cc[:, :, LOOKAHEAD, :, 0] += 1.0 - a[:, :, None]
    # (B, T, ORDER, F, 2) -> (B, F, 2, ORDER, T)
    ct = cc.transpose(0, 3, 4, 2, 1)
    CX = np.empty((B, F, 3, ORDER, T), np.float16)
    CX[:, :, 0] = ct[:, :, 0]                                   # cr
    CX[:, :, 1] = ct[:, :, 1]                                   # ci
    CX[:, :, 2] = ct[:, :, 0] + ct[:, :, 1]                     # cs = cr + ci
    CX = CX.reshape(B * F, 15 * T)
    return X, CX


def run_on_cores(spec, coefs, alpha, trace=False):
    from concourse import bass_utils

    nc = _get_program()
    X, CX = _host_prep(spec, coefs, alpha)
    in_maps = []
    for c in range(NCORES):
        sl = slice(c * ROWS, (c + 1) * ROWS)
        in_maps.append(
            {
                "x_t": np.ascontiguousarray(X[sl].reshape(NRT, 128, 2 * TP)),
                "cx_t": np.ascontiguousarray(CX[sl].reshape(NRT, 128, 15 * T)),
            }
        )
    res = bass_utils.run_bass_kernel_spmd(
        nc, in_maps, core_ids=list(range(NCORES)), trace=trace
    )
    full = np.array(spec, dtype=np.float32, copy=True)  # f>=96 passthrough on host
    outs = np.concatenate(
        [res.results[c]["out_t"].reshape(ROWS, 2, T) for c in range(NCORES)]
    )                                                   # (B*F, 2, T)
    blend = outs.reshape(B, F, 2, T).transpose(0, 3, 1, 2).astype(np.float32)
    full[:, 0, :, :F, :] = blend
    return full, res


def kernel(spec, coefs, alpha):
    spec = np.asarray(spec, dtype=np.float32)
    coefs = np.asarray(coefs, dtype=np.float32)
    alpha = np.asarray(alpha, dtype=np.float32)
    full, _ = run_on_cores(spec, coefs, alpha, trace=False)
    return full


# revision 10
# speedup vs baseline: 1.0691x; 1.0691x over previous
"""Trainium2 Bass kernel for nn_DfOpCoefLoop (deep-filter complex FIR + alpha blend).

Reference semantics (per batch b, time t, freq bin f < 96):
    spec_f[t,f] = sum_{i=0..4} x[t+i-2, f] * coefs[t,i,f]      (complex MAC, zero-padded in t)
    out[t,f]    = alpha[t] * spec_f[t,f] + (1-alpha[t]) * x[t,f]
    out[t,f]    = spec[t,f]                                    (f >= 96 passthrough)

Device-side transformations:
  - Alpha folding (host): c' = alpha*c with (1-alpha) added to the real coef of
    the center tap (i=2) -> pure 5-tap complex FIR on device.
  - Layout: rows = (batch, freq) pairs -> 384 rows/core = 3 tiles of 128
    partitions; time along the free dim; tap shifts are free-dim offsets, so x
    is loaded once.
  - fp16 TensorTensor everywhere (2x_1p mode, ~0.55 ns/elem on DVE). All work
    stays on DVE: GPSIMD is 4.6x slower AND poisons concurrent DVE throughput
    (SBUF contention); TensorReduce/STT never get the fp16 fast mode.
  - Karatsuba 3-mult complex tap product (cs = cr+ci folded on host):
        m1_i = xr_i*cr_i   m2_i = xi_i*ci_i   m3_i = (xr_i+xi_i)*cs_i
        re = R1 - R2       im = R3 - R1 - R2      (R_g = sum_i m_g_i)
  - Products as big [128, 5, TT] TensorTensor instrs via overlapping APs
    (tap dim stride 1 over the same row) -- verified to run at the full rate.
  - Tap sums via a log tree of TensorTensor adds over all 3 groups at once.
  - Each row-tile is split into 2 time-chunks (6 pipeline units/core) with a
    chunk-contiguous DRAM layout, so the first products start after ~1MB of
    DMA and the drain tail is half a chunk. DMAs go whole-tile (no partition
    slicing -- measured slower) on the two issue queues in need-order.
"""

import numpy as np

ORDER = 5
LOOKAHEAD = 2
F = 96
T = 1000
NCH = 2                     # time chunks per row-tile
TT = T // NCH               # 500 output samples per chunk
TPu = TT + ORDER - 1        # 504 padded samples per chunk
B = 32
NCORES = 8
BPC = B // NCORES           # 4 batches per core
ROWS = BPC * F              # 384 rows per core
NRT = ROWS // 128           # 3 row-tiles per core

_CACHE = {}


def _build_program():
    import concourse.bacc as bacc
    import concourse.mybir as mybir
    import concourse.tile as tile
    from concourse.ap import AP

    nc = bacc.Bacc("TRN2", target_bir_lowering=False, debug=False)
    dt = mybir.dt.float16
    mul = mybir.AluOpType.mult

    x_t = nc.dram_tensor("x_t", [NRT, NCH, 128, 2 * TPu], dt, kind="ExternalInput").ap()
    cx_t = nc.dram_tensor("cx_t", [NRT, NCH, 128, 15 * TT], dt, kind="ExternalInput").ap()
    out_t = nc.dram_tensor("out_t", [NRT, NCH, 128, 2 * TT], dt, kind="ExternalOutput").ap()

    def taps(tile_ap, row_elems, off):
        """Overlapping [128, 5, TT] view: (p, i, t) -> base + off + i + t."""
        return AP(tile_ap.tensor, tile_ap.offset + off, [[row_elems, 128], [1, ORDER], [1, TT]])

    with tile.TileContext(nc) as tc:
        with (
            tc.tile_pool(name="xp", bufs=3) as xp,
            tc.tile_pool(name="sp", bufs=3) as sp,
            tc.tile_pool(name="cp", bufs=3) as cp,
            tc.tile_pool(name="pp", bufs=1) as pp,
            tc.tile_pool(name="ap_", bufs=1) as ap_,
            tc.tile_pool(name="bp", bufs=1) as bp,
            tc.tile_pool(name="rp", bufs=1) as rp,
            tc.tile_pool(name="obp", bufs=2) as obp,
        ):
            for rt in range(NRT):
                for h in range(NCH):
                    x = xp.tile([128, 2 * TPu], dt, name="x")
                    s = sp.tile([128, TPu], dt, name="s")
                    c = cp.tile([128, 15, TT], dt, name="c")
                    nc.sync.dma_start(x[:], x_t[rt, h])
                    nc.sync.dma_start(c[:, 0:5], cx_t[rt, h, :, : 5 * TT])
                    nc.scalar.dma_start(c[:, 5:10], cx_t[rt, h, :, 5 * TT : 10 * TT])
                    nc.scalar.dma_start(c[:, 10:15], cx_t[rt, h, :, 10 * TT :])

                    p = pp.tile([128, 3, ORDER, TT], dt, name="p")
                    nc.vector.tensor_add(s[:], x[:, 0:TPu], x[:, TPu : 2 * TPu])
                    nc.vector.tensor_tensor(p[:, 0], taps(x[:], 2 * TPu, 0), c[:, 0:5], op=mul)
                    nc.vector.tensor_tensor(p[:, 1], taps(x[:], 2 * TPu, TPu), c[:, 5:10], op=mul)
                    nc.vector.tensor_tensor(p[:, 2], taps(s[:], TPu, 0), c[:, 10:15], op=mul)

                    a = ap_.tile([128, 3, 2, TT], dt, name="a")
                    bb = bp.tile([128, 3, TT], dt, name="bb")
                    r = rp.tile([128, 3, TT], dt, name="r")
                    ti = rp.tile([128, TT], dt, name="ti")
                    ob = obp.tile([128, 2 * TT], dt, name="ob")
                    nc.vector.tensor_add(a[:], p[:, :, 0:2], p[:, :, 2:4])
                    nc.vector.tensor_add(bb[:], a[:, :, 0], a[:, :, 1])
                    nc.vector.tensor_add(r[:], bb[:], p[:, :, 4])
                    nc.vector.tensor_sub(ob[:, 0:TT], r[:, 0], r[:, 1])
                    nc.sync.dma_start(out_t[rt, h, :, 0:TT], ob[:, 0:TT])
                    nc.vector.tensor_sub(ti[:], r[:, 2], r[:, 0])
                    nc.vector.tensor_sub(ob[:, TT : 2 * TT], ti[:], r[:, 1])
                    nc.scalar.dma_start(out_t[rt, h, :, TT : 2 * TT], ob[:, TT : 2 * TT])
    nc.compile()
    return nc


def _get_program():
    if "nc" not in _CACHE:
        _CACHE["nc"] = _build_program()
    return _CACHE["nc"]


def _host_prep(spec, coefs, alpha):
    """Build the chunked (row, free) fp16 device layouts for all 32 batches."""
    x = np.asarray(spec[:, 0, :, :F, :], dtype=np.float32)      # (B, T, F, 2)
    Xp = np.zeros((B, F, 2, T + ORDER - 1), np.float16)
    Xp[:, :, :, LOOKAHEAD : LOOKAHEAD + T] = x.transpose(0, 2, 3, 1)
    # chunked x with halo: (BF, NCH, 2, TPu)
    Xc = np.empty((B * F, NCH, 2, TPu), np.float16)
    Xpr = Xp.reshape(B * F, 2, T + ORDER - 1)
    for h in range(NCH):
        Xc[:, h] = Xpr[:, :, h * TT : h * TT + TPu]

    a = np.asarray(alpha, dtype=np.float32)[:, :, 0]            # (B, T)
    cc = np.asarray(coefs, dtype=np.float32) * a[:, :, None, None, None]
    cc[:, :, LOOKAHEAD, :, 0] += 1.0 - a[:, :, None]
    # (B, T, ORDER, F, 2) -> (BF, 3, ORDER, T)
    ct = cc.transpose(0, 3, 4, 2, 1)
    bf = B * F
    CX = np.empty((bf, 3, ORDER, T), np.float16)
    CX[:, 0] = ct[:, :, 0].reshape(bf, ORDER, T)                # cr
    CX[:, 1] = ct[:, :, 1].reshape(bf, ORDER, T)                # ci
    CX[:, 2] = CX[:, 0] + CX[:, 1]                              # cs
    # chunk the t axis: (BF, NCH, 15, TT)
    CXc = np.ascontiguousarray(
        CX.reshape(bf, 15, NCH, TT).transpose(0, 2, 1, 3)
    )
    return Xc, CXc


def run_on_cores(spec, coefs, alpha, trace=False):
    from concourse import bass_utils

    nc = _get_program()
    Xc, CXc = _host_prep(spec, coefs, alpha)
    in_maps = []
    for c in range(NCORES):
        sl = slice(c * ROWS, (c + 1) * ROWS)
        in_maps.append(
            {
                "x_t": np.ascontiguousarray(
                    Xc[sl].reshape(NRT, 128, NCH, 2 * TPu).transpose(0, 2, 1, 3)
                ),
                "cx_t": np.ascontiguousarray(
                    CXc[sl].reshape(NRT, 128, NCH, 15 * TT).transpose(0, 2, 1, 3)
                ),
            }
        )
    res = bass_utils.run_bass_kernel_spmd(
        nc, in_maps, core_ids=list(range(NCORES)), trace=trace
    )
    full = np.array(spec, dtype=np.float32, copy=True)  # f>=96 passthrough on host
    outs = np.concatenate(
        [
            res.results[c]["out_t"].reshape(NRT, NCH, 128, 2, TT)
            .transpose(0, 2, 3, 1, 4)
            .reshape(ROWS, 2, T)
            for c in range(NCORES)
        ]
    )                                                   # (B*F, 2, T)
    blend = outs.reshape(B, F, 2, T).transpose(0, 3, 1, 2).astype(np.float32)
    full[:, 0, :, :F, :] = blend
    return full, res


def kernel(spec, coefs, alpha):
    spec = np.asarray(spec, dtype=np.float32)
    coefs = np.asarray(coefs, dtype=np.float32)
    alpha = np.asarray(alpha, dtype=np.float32)
    full, _ = run_on_cores(spec, coefs, alpha, trace=False)
    return full
